# Initial kernel scaffold
#
"""Trainium2 Bass kernel for nn_DisLoss: loss = sum(x * dist_to_argmax(x)) / b.

x: (128, 512, 512) f32. Data-parallel over 8 NeuronCores: 16 batch images per
core. Per image on-device:
  1. DVE 3D reduce-max -> per-row maxes (128 partitions x 4 rows each)
  2. tiny cross-partition ops (GPSIMD partition_all_reduce) -> global max M, row cy
  3. indirect-DMA gather of the winning row, spread over 16 partitions x 64
     (row duplicated twice); masked iota-dot -> column cx
  4. ACT: colsq = (j-cx)^2, rowsq = (r-cy)^2, dist = sqrt(colsq + rowsq)
  5. fused multiply+accumulate (scalar_tensor_tensor accum_out) of x*dist,
     split across DVE and GPSIMD -> per-partition partials
Host sums the (128, 64) per-core partials in float64 and divides by b.
"""

import numpy as np

B_FULL = 128
H = 512
W = 512
N_CORES = 8
B_CORE = B_FULL // N_CORES  # 16 images per core
T = 4                       # rows per partition (512 rows / 128 partitions)
CHUNK = W                   # free-dim elements per row chunk
ROWP = 16                   # partitions used for the gathered-row scan
ROWF = 64                   # free elems per partition in the row scan (2*512/16)

# z-chunk split: chunk t -> engine ("dve" or "gps")
Z_SPLIT = ("dve", "gps", "gps", "gps")

_CACHE = {}


def _consts():
    cols = np.broadcast_to(np.arange(W, dtype=np.float32), (128, W)).copy()
    r4c = (4 * np.arange(128, dtype=np.float32)[:, None]
           + np.arange(T, dtype=np.float32)[None, :]).copy()
    r4cn = (-r4c).copy()
    lin = (64 * np.arange(ROWP, dtype=np.float32)[:, None]
           + np.arange(ROWF, dtype=np.float32)[None, :])
    iotawn = (-np.mod(lin, 512) / 2.0).astype(np.float32)
    ones2 = np.ones((1, 2), dtype=np.float32)
    return {"COLS": cols, "R4C": r4c, "R4CN": r4cn, "IOTAWN": iotawn,
            "ONES2": ones2}


def build_program(debug=False):
    import concourse.bass as bass
    import concourse.mybir as mybir
    import concourse.bass_isa as bass_isa
    from concourse.tile import TileContext

    f32 = mybir.dt.float32
    u32 = mybir.dt.uint32
    Alu = mybir.AluOpType
    Act = mybir.ActivationFunctionType

    nc = bass.Bass("TRN2", target_bir_lowering=False, debug=False)

    x_d = nc.dram_tensor("x", [B_CORE, H, W], f32, kind="ExternalInput")
    cols_d = nc.dram_tensor("COLS", [128, W], f32, kind="ExternalInput")
    r4c_d = nc.dram_tensor("R4C", [128, T], f32, kind="ExternalInput")
    r4cn_d = nc.dram_tensor("R4CN", [128, T], f32, kind="ExternalInput")
    iotawn_d = nc.dram_tensor("IOTAWN", [ROWP, ROWF], f32, kind="ExternalInput")
    ones2_d = nc.dram_tensor("ONES2", [1, 2], f32, kind="ExternalInput")
    out_d = nc.dram_tensor("partials", [128, B_CORE * T], f32,
                           kind="ExternalOutput")
    if debug:
        dbg_d = nc.dram_tensor("dbg", [B_CORE, 4], f32, kind="ExternalOutput")

    x_ap = x_d.ap()
    x_rows = x_ap.rearrange("b h w -> (b h) w")  # (8192, 512) for row gather

    with TileContext(nc) as tc:
        with (
            tc.tile_pool(name="consts", bufs=1) as consts,
            tc.tile_pool(name="xs", bufs=4) as xs,
            tc.tile_pool(name="work", bufs=6) as work,
            tc.tile_pool(name="small", bufs=3) as small,
            tc.tile_pool(name="rows", bufs=3) as rows,
        ):
            cols_t = consts.tile([128, W], f32)
            nc.sync.dma_start(out=cols_t, in_=cols_d.ap())
            r4c_t = consts.tile([128, T], f32)
            nc.sync.dma_start(out=r4c_t, in_=r4c_d.ap())
            r4cn_t = consts.tile([128, T], f32)
            nc.sync.dma_start(out=r4cn_t, in_=r4cn_d.ap())
            iotawn_t = consts.tile([ROWP, ROWF], f32)
            nc.sync.dma_start(out=iotawn_t, in_=iotawn_d.ap())
            ones2_t = consts.tile([1, 2], f32)
            nc.sync.dma_start(out=ones2_t, in_=ones2_d.ap())

            partials_t = consts.tile([128, B_CORE * T], f32)
            if debug:
                dbg_t = consts.tile([B_CORE, 4], f32)

            for b in range(B_CORE):
                # --- load image b: partition p holds rows 4p..4p+3 ---
                x_t = xs.tile([128, T, W], f32)
                nc.sync.dma_start(
                    out=x_t,
                    in_=x_ap[b].rearrange("(p t) w -> p t w", p=128),
                )

                # --- pass A: per-row maxes, then global max M ---
                rowmax = small.tile([128, T], f32)
                nc.vector.reduce_max(rowmax, x_t, axis=mybir.AxisListType.X)
                pmax = small.tile([128, 1], f32)
                nc.vector.reduce_max(pmax, rowmax, axis=mybir.AxisListType.X)
                m_all = small.tile([128, 1], f32)
                nc.gpsimd.partition_all_reduce(
                    m_all, pmax, channels=128, reduce_op=bass_isa.ReduceOp.max)

                # --- locate row: accum of (rowmax==M) * (-(4p+t)) -> -cy ---
                oh4 = small.tile([128, T], f32)
                negcy_p = small.tile([128, 1], f32)
                nc.vector.scalar_tensor_tensor(
                    oh4, rowmax, m_all, r4cn_t,
                    op0=Alu.is_equal, op1=Alu.mult, accum_out=negcy_p)
                negcy = small.tile([128, 1], f32)
                nc.gpsimd.partition_all_reduce(
                    negcy, negcy_p, channels=128,
                    reduce_op=bass_isa.ReduceOp.add)

                # --- gather winning row (twice) spread over 16 partitions ---
                # global row index = 512*b + cy  (as uint32, duplicated)
                cyu = small.tile([1, 2], u32)
                nc.vector.tensor_scalar(
                    cyu, ones2_t, float(512 * b), negcy[0:1, 0:1],
                    op0=Alu.mult, op1=Alu.subtract)
                rowbuf = rows.tile([ROWP, ROWF], f32)
                nc.gpsimd.indirect_dma_start(
                    out=rowbuf,
                    out_offset=None,
                    in_=x_rows,
                    in_offset=bass.IndirectOffsetOnAxis(ap=cyu[:], axis=0),
                )

                # --- locate column: accum of (row==M) * (-j/2) -> -cx ---
                ohr = rows.tile([ROWP, ROWF], f32)
                negcx_p = small.tile([ROWP, 1], f32)
                nc.vector.scalar_tensor_tensor(
                    ohr, rowbuf, m_all[0:ROWP, 0:1], iotawn_t,
                    op0=Alu.is_equal, op1=Alu.mult, accum_out=negcx_p)
                negcx16 = small.tile([ROWP, 1], f32)
                nc.gpsimd.partition_all_reduce(
                    negcx16, negcx_p, channels=ROWP,
                    reduce_op=bass_isa.ReduceOp.add)
                negcx = small.tile([128, 1], f32)
                nc.gpsimd.partition_broadcast(negcx, negcx16[0:1, 0:1])

                # --- distances ---
                colsq = work.tile([128, W], f32)
                nc.scalar.activation(colsq, cols_t, Act.Square, bias=negcx)
                rowsq = small.tile([128, T], f32)
                nc.scalar.activation(rowsq, r4c_t, Act.Square, bias=negcy)

                for t in range(T):
                    dist_t = work.tile([128, W], f32, tag="dist")
                    nc.scalar.activation(
                        dist_t, colsq, Act.Sqrt, bias=rowsq[:, t:t + 1])
                    z_t = work.tile([128, W], f32, tag="z")
                    eng = nc.vector if Z_SPLIT[t] == "dve" else nc.gpsimd
                    eng.scalar_tensor_tensor(
                        z_t, dist_t, 1.0, x_t[:, t, :],
                        op0=Alu.mult, op1=Alu.mult,
                        accum_out=partials_t[:, T * b + t:T * b + t + 1])

                if debug:
                    nc.vector.tensor_scalar(
                        dbg_t[b:b + 1, 0:1], ones2_t[0:1, 0:1],
                        m_all[0:1, 0:1], None, op0=Alu.mult)
                    nc.vector.tensor_scalar(
                        dbg_t[b:b + 1, 1:2], ones2_t[0:1, 0:1],
                        negcy[0:1, 0:1], None, op0=Alu.mult)
                    nc.vector.tensor_scalar(
                        dbg_t[b:b + 1, 2:3], ones2_t[0:1, 0:1],
                        negcx[0:1, 0:1], None, op0=Alu.mult)

            nc.sync.dma_start(out=out_d.ap(), in_=partials_t)
            if debug:
                nc.sync.dma_start(out=dbg_d.ap(), in_=dbg_t)

    return nc


def kernel(x: np.ndarray) -> np.ndarray:
    from concourse import bass_utils

    key = "nc"
    if key not in _CACHE:
        _CACHE[key] = build_program(debug=False)
    nc = _CACHE[key]

    x = np.ascontiguousarray(x, dtype=np.float32)
    shards = x.reshape(N_CORES, B_CORE, H, W)
    consts = _consts()
    in_maps = [dict(consts, x=shards[i]) for i in range(N_CORES)]
    res = bass_utils.run_bass_kernel_spmd(
        nc, in_maps, core_ids=list(range(N_CORES)))
    total = 0.0
    for r in res.results:
        total += r["partials"].astype(np.float64).sum()
    return np.float32(total / B_FULL)


# revision 18
# speedup vs baseline: 1.0558x; 1.0558x over previous
"""Trainium2 Bass kernel for nn_DisLoss: loss = sum(x * dist_to_argmax(x)) / b.

x: (128, 512, 512) f32. Data-parallel over 8 NeuronCores: 16 images per core.
Per image on-device:
  1. DVE 3D reduce-max -> per-row maxes (partition p holds rows 4p..4p+3)
  2. PE transpose + tiny DVE reduce -> global max M; PE ones-matmuls do all
     cross-partition sums/broadcasts (GPSIMD stays on its default ucode lib)
  3. indirect-DMA gather of the winning row spread over 16 partitions x 64
     (row appears twice); masked iota-dot -> column -cx
  4. ACT: colsq=(j-cx)^2, rowsq=(r-cy)^2, dist_t=sqrt(colsq+rowsq_t)
  5. z = x*dist: chunk t=0 fused mul+accum on DVE -> partials[:, b];
     chunks t=1..3 multiplied on GPSIMD, column-summed into one PSUM row by
     accumulating PE matmuls (lhsT = ones column)
Host sums partials + colsums in float64 and divides by b.
"""

import numpy as np

B_FULL = 128
H = 512
W = 512
N_CORES = 8
B_CORE = B_FULL // N_CORES  # 16 images per core
T = 4                       # rows per partition
ROWP = 8                    # partitions for the gathered-row scan
ROWF = 64                   # free elems per partition in the row scan

_CACHE = {}


def _consts():
    cols = np.broadcast_to(np.arange(W, dtype=np.float32), (128, W)).copy()
    r4c = (4 * np.arange(128, dtype=np.float32)[:, None]
           + np.arange(T, dtype=np.float32)[None, :]).copy()
    r4enc8 = (8.0 * (4096.0 - r4c)).copy()
    lin = (64 * np.arange(ROWP, dtype=np.float32)[:, None]
           + np.arange(ROWF, dtype=np.float32)[None, :])
    colenc = (512.0 - lin).astype(np.float32)
    pidx8 = np.arange(ROWP, dtype=np.float32)[:, None].copy()
    ones2 = np.ones((1, 2), dtype=np.float32)
    pec = np.concatenate([np.eye(128, dtype=np.float32),
                          np.ones((128, 128), dtype=np.float32)], axis=1)
    return {"COLS": cols, "R4C": r4c, "R4ENC8": r4enc8, "COLENC": colenc,
            "PIDX8": pidx8, "ONES2": ones2, "PEC": pec}


def build_program(debug=False, b_core=None, no_indirect=False,
                  no_psum_acc=False):
    import concourse.bass as bass
    import concourse.bacc as bacc
    import concourse.mybir as mybir
    from concourse.tile import TileContext

    nb = b_core or B_CORE
    f32 = mybir.dt.float32
    u32 = mybir.dt.uint32
    Alu = mybir.AluOpType
    Act = mybir.ActivationFunctionType

    nc = bacc.Bacc("TRN2", target_bir_lowering=False, debug=False)

    x_d = nc.dram_tensor("x", [nb, H, W], f32, kind="ExternalInput")
    cols_d = nc.dram_tensor("COLS", [128, W], f32, kind="ExternalInput")
    r4c_d = nc.dram_tensor("R4C", [128, T], f32, kind="ExternalInput")
    r4enc8_d = nc.dram_tensor("R4ENC8", [128, T], f32, kind="ExternalInput")
    pidx8_d = nc.dram_tensor("PIDX8", [ROWP, 1], f32, kind="ExternalInput")
    colenc_d = nc.dram_tensor("COLENC", [ROWP, ROWF], f32, kind="ExternalInput")
    ones2_d = nc.dram_tensor("ONES2", [1, 2], f32, kind="ExternalInput")
    pec_d = nc.dram_tensor("PEC", [128, 256], f32, kind="ExternalInput")
    out_d = nc.dram_tensor("partials", [128, nb], f32, kind="ExternalOutput")
    csum_d = nc.dram_tensor("colsums", [1, W], f32, kind="ExternalOutput")
    if debug:
        dbg_d = nc.dram_tensor("dbg", [1, 4 * nb], f32, kind="ExternalOutput")

    x_ap = x_d.ap()
    x_rows = x_ap.rearrange("b h (s i) -> (b h s) i", i=ROWF)  # 64-elem sub-rows

    with TileContext(nc) as tc:
        with (
            tc.tile_pool(name="consts", bufs=1) as consts,
            tc.tile_pool(name="xs", bufs=6) as xs,
            tc.tile_pool(name="work", bufs=6) as work,
            tc.tile_pool(name="small", bufs=6) as small,
            tc.tile_pool(name="rows", bufs=4) as rows,
            tc.tile_pool(name="ps", bufs=3, space="PSUM") as ps,
            tc.tile_pool(name="pacc", bufs=1, space="PSUM") as pacc,
        ):
            cols_t = consts.tile([128, W], f32)
            nc.sync.dma_start(out=cols_t, in_=cols_d.ap())
            r4c_t = consts.tile([128, T], f32)
            nc.sync.dma_start(out=r4c_t, in_=r4c_d.ap())
            r4enc8_t = consts.tile([128, T], f32)
            nc.sync.dma_start(out=r4enc8_t, in_=r4enc8_d.ap())
            pidx8_t = consts.tile([ROWP, 1], f32)
            nc.sync.dma_start(out=pidx8_t, in_=pidx8_d.ap())
            colenc_t = consts.tile([ROWP, ROWF], f32)
            nc.sync.dma_start(out=colenc_t, in_=colenc_d.ap())
            ones2_t = consts.tile([1, 2], f32)
            nc.sync.dma_start(out=ones2_t, in_=ones2_d.ap())
            pec_t = consts.tile([128, 256], f32)
            nc.sync.dma_start(out=pec_t, in_=pec_d.ap())
            ident_t = pec_t[:, 0:128]
            ones_t = pec_t[:, 128:256]

            # prime PE on the const tile so later PE ops carry only one wait
            prime_ps = ps.tile([1, 128], f32, tag="tpose")
            nc.tensor.transpose(prime_ps, pec_t[:, 128:129], ident_t)

            partials_t = consts.tile([128, nb], f32)
            colsums_ps = pacc.tile([1, W], f32)
            if debug:
                dbg_t = consts.tile([1, 4 * nb], f32)
                nc.vector.memset(dbg_t, 0.0)

            for b in range(nb):
                # --- load image b: partition p holds rows 4p..4p+3 ---
                x_t = xs.tile([128, T, W], f32)
                nc.sync.dma_start(
                    out=x_t,
                    in_=x_ap[b].rearrange("(p t) w -> p t w", p=128),
                )

                # --- per-row maxes -> per-partition max -> global max M ---
                rowmax = small.tile([128, T], f32)
                nc.vector.reduce_max(rowmax, x_t, axis=mybir.AxisListType.X)
                pmax = small.tile([128, 1], f32)
                nc.vector.reduce_max(pmax, rowmax, axis=mybir.AxisListType.X)
                pmax_ps = ps.tile([1, 128], f32, tag="tpose")
                nc.tensor.transpose(pmax_ps, pmax, ident_t)
                m_sb = small.tile([1, 1], f32)
                nc.vector.reduce_max(m_sb, pmax_ps, axis=mybir.AxisListType.X)
                sc_ps = ps.tile([128, 3], f32, tag="sc")
                m_ps = sc_ps[:, 0:1]
                nc.tensor.matmul(m_ps, ones_t[0:1, :], m_sb)

                # --- locate row: max of (rowmax==M)*(8*(4096-r)) -> first row ---
                e4 = small.tile([128, T], f32)
                junk1 = small.tile([128, 1], f32)
                nc.vector.scalar_tensor_tensor(
                    e4, rowmax, m_ps, r4enc8_t,
                    op0=Alu.is_equal, op1=Alu.mult, accum_out=junk1)
                epmax = small.tile([128, 1], f32)
                nc.vector.reduce_max(epmax, e4, axis=mybir.AxisListType.X)
                ep_ps = ps.tile([1, 128], f32, tag="tpose")
                nc.tensor.transpose(ep_ps, epmax, ident_t)
                e_sb = small.tile([1, 1], f32)
                nc.vector.reduce_max(e_sb, ep_ps, axis=mybir.AxisListType.X)
                nc.tensor.matmul(sc_ps[:, 1:2], ones_t[0:1, :], e_sb)

                # --- gather the winning row, 64 elems per partition ---
                cyu = small.tile([ROWP, 1], u32)
                nc.vector.tensor_scalar(
                    cyu, pidx8_t, sc_ps[0:ROWP, 1:2], float(8 * 4096 + 8 * H * b),
                    op0=Alu.subtract, op1=Alu.add)
                rowbuf = rows.tile([ROWP, ROWF], f32)
                if no_indirect:
                    nc.sync.dma_start(out=rowbuf, in_=x_ap[b, 0:ROWP, 0:ROWF])
                else:
                    nc.gpsimd.indirect_dma_start(
                        out=rowbuf,
                        out_offset=None,
                        in_=x_rows,
                        in_offset=bass.IndirectOffsetOnAxis(ap=cyu[:], axis=0),
                    )

                # --- locate column: max of (row==M)*(512-j) -> first col ---
                ohr = rows.tile([ROWP, ROWF], f32)
                junkr = small.tile([ROWP, 1], f32)
                nc.vector.scalar_tensor_tensor(
                    ohr, rowbuf, m_ps[0:ROWP, 0:1], colenc_t,
                    op0=Alu.is_equal, op1=Alu.mult, accum_out=junkr)
                ecp = small.tile([ROWP, 1], f32)
                nc.vector.reduce_max(ecp, ohr, axis=mybir.AxisListType.X)
                ec_ps = ps.tile([1, ROWP], f32, tag="tpose")
                nc.tensor.transpose(ec_ps, ecp, ident_t[0:ROWP, 0:ROWP])
                ec_sb = small.tile([1, 1], f32)
                nc.vector.reduce_max(ec_sb, ec_ps, axis=mybir.AxisListType.X)
                nc.tensor.matmul(sc_ps[:, 2:3], ones_t[0:1, :], ec_sb)
                nsb = small.tile([128, 2], f32)
                nc.vector.tensor_copy(nsb, sc_ps[:, 1:3])
                negcy_sb = small.tile([128, 1], f32)
                nc.vector.tensor_scalar(
                    negcy_sb, nsb[:, 0:1], 0.125, -4096.0,
                    op0=Alu.mult, op1=Alu.add)
                negcx_sb = small.tile([128, 1], f32)
                nc.vector.tensor_scalar(
                    negcx_sb, nsb[:, 1:2], -512.0, None, op0=Alu.add)

                # --- distances ---
                colsq = work.tile([128, W], f32)
                nc.scalar.activation(colsq, cols_t, Act.Square,
                                     bias=negcx_sb)
                rowsq = small.tile([128, T], f32)
                nc.scalar.activation(rowsq, r4c_t, Act.Square,
                                     bias=negcy_sb)

                for t in range(T):
                    dist_t = work.tile([128, W], f32, tag="dist")
                    nc.scalar.activation(
                        dist_t, colsq, Act.Sqrt, bias=rowsq[:, t:t + 1])
                    if t == 0:
                        z_t = work.tile([128, W], f32, tag="zd")
                        nc.vector.scalar_tensor_tensor(
                            z_t, dist_t, 1.0, x_t[:, t, :],
                            op0=Alu.mult, op1=Alu.mult,
                            accum_out=partials_t[:, b:b + 1])
                    else:
                        z_t = work.tile([128, W], f32, tag="zg")
                        nc.gpsimd.tensor_tensor(
                            z_t, dist_t, x_t[:, t, :], op=Alu.mult)
                        if no_psum_acc:
                            nc.tensor.matmul(
                                colsums_ps, ones_t[:, 0:1], z_t,
                                start=True, stop=True,
                                skip_group_check=True)
                        else:
                            nc.tensor.matmul(
                                colsums_ps, ones_t[:, 0:1], z_t,
                                start=(b == 0 and t == 1),
                                stop=(b == nb - 1 and t == T - 1),
                                skip_group_check=True)

                if debug:
                    nc.vector.tensor_scalar(
                        dbg_t[0:1, 4 * b:4 * b + 1], ones2_t[0:1, 0:1],
                        m_ps[0:1, 0:1], None, op0=Alu.mult)
                    nc.vector.tensor_scalar(
                        dbg_t[0:1, 4 * b + 1:4 * b + 2], ones2_t[0:1, 0:1],
                        nsb[0:1, 0:1], None, op0=Alu.mult)
                    nc.vector.tensor_scalar(
                        dbg_t[0:1, 4 * b + 2:4 * b + 3], ones2_t[0:1, 0:1],
                        nsb[0:1, 1:2], None, op0=Alu.mult)

            colsums_sb = consts.tile([1, W], f32)
            nc.vector.tensor_copy(colsums_sb, colsums_ps)
            nc.sync.dma_start(out=out_d.ap(), in_=partials_t)
            nc.sync.dma_start(out=csum_d.ap(), in_=colsums_sb)
            if debug:
                nc.sync.dma_start(out=dbg_d.ap(), in_=dbg_t)

    nc.compile()
    return nc


def kernel(x: np.ndarray) -> np.ndarray:
    from concourse import bass_utils

    key = "nc"
    if key not in _CACHE:
        _CACHE[key] = build_program(debug=False)
    nc = _CACHE[key]

    x = np.ascontiguousarray(x, dtype=np.float32)
    shards = x.reshape(N_CORES, B_CORE, H, W)
    consts = _consts()
    in_maps = [dict(consts, x=shards[i]) for i in range(N_CORES)]
    res = bass_utils.run_bass_kernel_spmd(
        nc, in_maps, core_ids=list(range(N_CORES)))
    total = 0.0
    for r in res.results:
        total += r["partials"].astype(np.float64).sum()
        total += r["colsums"].astype(np.float64).sum()
    return np.float32(total / B_FULL)
